# revision 19
# baseline (speedup 1.0000x reference)
"""Self-contained 8-core Trainium2 Bass kernel for MultiHeadAttention.

Problem: B=2, S=2048, D=1024, H=16 heads (hd=64), f32, self-attention
(no mask), eval mode (dropout = identity).

Sharding: data-parallel over B (2) x tensor-parallel over heads (4 groups
of 4 heads) = 8 cores. Each core computes, for its batch b and its 4
heads: Q/K/V projections (column-sliced), attention, and a partial
output projection (row-sliced Wo). Host sums the 4 partials per batch
and adds the (bv @ Wo + bo) correction (bv never enters the kernel:
ctx rows sum probs to 1, so (ctx+bv) @ Wo = ctx @ Wo + bv @ Wo).

Algebraic simplifications (exact):
  - bk dropped: softmax over k is invariant to the per-q constant Q.bk.
  - softmax without max subtraction (scores bounded, exp safe in f32).
  - bq folded into Q^T as a per-partition masked bias.
  - row normalization deferred past P@V (scale ctx, not probs); row sums
    obtained free via an appended ones-column in V.

v2 changes vs the original baseline (291.7us HW):
  - all matmul operands bf16 (same PE rate as f32r at these tile sizes,
    but half the DMA / SBUF / DVE traffic); PSUM accumulation stays f32.
    Output partials shipped bf16, summed f32 on host. Measured numpy
    end-to-end rel err ~5e-3 vs the 2e-2 gate.
  - x^T loaded in 512-column slabs so projections start after ~1.5MB of
    DMA instead of 8MB; per-slab projection work is emitted right after
    each slab's DMA, and attention(pair0, qc0) is woven into the slab
    window (its r-loop consumes k-position tiles in slab order).
  - PE kept continuously busy: deferred projection + output-projection
    matmuls are interleaved as fillers into the ACT(exp)-paced attention
    stretches; a short warmup matmul run ramps the PE p-state before the
    first real matmul.
  - softmax normalization drains PSUM via an early copy to SBUF so the
    2 ctx PSUM banks recycle without stalling the next attention call.

Layouts per core: x^T [D, S] slabs; K^T/Q^T per head-pair with the two
heads stacked on partitions; scores^T computed as K-chunk @ Q^T with
k-positions on output partitions (one matmul per (qc, r) covering both
heads via masked Q); exp on ACT (f32 PSUM -> bf16 SBUF); PV accumulates
ctx^T[hd+1, q] over r in PSUM with the ones-row giving the softmax
denominator; out projection contracts head dims with Wo as the moving
operand.
"""

import sys

sys.path.insert(0, "/opt/trn_rl_repo")

import numpy as np
import ml_dtypes

B, S, D, H, HD = 2, 2048, 1024, 16, 64
HPC = 4  # heads per core
NCORES = 8
DC = D // 128  # 8 contraction chunks
ST = S // 128  # 16 s-tiles
QCW = 512  # q chunk width == slab width
QC = S // QCW  # 4 q chunks == 4 slabs
KT = S // 128  # 16 k tiles

_CACHE = {}


def _build(repeat=1, warmup=10):
    import concourse.bass as bass  # noqa: F401
    import concourse.mybir as mybir
    import concourse.tile as tile
    from concourse import bacc
    from concourse.alu_op_type import AluOpType
    from concourse.library_config import attn as attn_lib

    F32 = mybir.dt.float32
    BF16 = mybir.dt.bfloat16
    AF = mybir.ActivationFunctionType

    nc = bacc.Bacc("TRN2", target_bir_lowering=False, debug=False)

    xt_d = nc.dram_tensor("xt", [D, S], BF16, kind="ExternalInput")
    wq_d = nc.dram_tensor("wq", [D, HPC * HD], BF16, kind="ExternalInput")
    wk_d = nc.dram_tensor("wk", [D, HPC * HD], BF16, kind="ExternalInput")
    wv_d = nc.dram_tensor("wv", [D, HPC * HD], BF16, kind="ExternalInput")
    wo_d = nc.dram_tensor("wo", [HPC * HD, D], BF16, kind="ExternalInput")
    mk_d = nc.dram_tensor("mk2", [128, 2], F32, kind="ExternalInput")
    bqm_d = nc.dram_tensor("bqm4", [128, 4], F32, kind="ExternalInput")
    out_d = nc.dram_tensor("out_p", [S, D], BF16, kind="ExternalOutput")

    with tile.TileContext(nc) as tc:
        nc.gpsimd.load_library(attn_lib)
        with (
            tc.tile_pool(name="wp", bufs=1) as wp,
            tc.tile_pool(name="xp", bufs=1) as xp,
            tc.tile_pool(name="qk", bufs=1) as qk,
            tc.tile_pool(name="vp", bufs=1) as vp,
            tc.tile_pool(name="ep", bufs=4) as ep,
            tc.tile_pool(name="cp", bufs=1) as cp,
            tc.tile_pool(name="c2", bufs=2) as c2p,
            tc.tile_pool(name="mp", bufs=2) as mp,
            tc.tile_pool(name="op", bufs=3) as op,
            tc.tile_pool(name="pp", bufs=2, space="PSUM") as pp,
        ):
            # persistent tiles (loaded / initialized once, reused each rep)
            wk_t = wp.tile([128, DC, HPC * HD], BF16, tag="wk")
            wq_t = wp.tile([128, DC, HPC * HD], BF16, tag="wq")
            wv_t = wp.tile([128, DC, HPC * HD], BF16, tag="wv")
            wo_t = wp.tile([128, 2, D], BF16, tag="wo")
            mk_t = wp.tile([128, 2], F32, tag="mk")
            bqm_t = wp.tile([128, 4], F32, tag="bqm")
            ones_b = wp.tile([128, 64], BF16, tag="ones")
            warm_in = wp.tile([128, QCW], BF16, tag="warmin")
            xt_t = xp.tile([128, DC, S], BF16, tag="xt")

            # DMA order: wk then slab0 gate the first projection; wq/etc
            # follow (qt runs ~1.7us after kt so they arrive in time).
            nc.sync.dma_start(wk_t[:], wk_d.rearrange("(c p) n -> p c n", p=128))
            nc.vector.memset(ones_b[:], 1.0)
            nc.vector.memset(warm_in[:], 0.0)

            import contextlib
            if repeat > 1:
                _engs = [mybir.EngineType.PE, mybir.EngineType.Activation,
                         mybir.EngineType.DVE, mybir.EngineType.SP,
                         mybir.EngineType.Pool]
                rep_ctx = tc.For_i(0, repeat, hint_engines=_engs, staggered_reset=True)
            else:
                rep_ctx = contextlib.nullcontext()
            with rep_ctx:
                # ---- x^T slab DMAs (+ wq/wv after slab0, wo after slab2)
                for s in range(QC):
                    qs = slice(s * QCW, (s + 1) * QCW)
                    for c in range(DC):
                        nc.sync.dma_start(
                            xt_t[:, c, qs], xt_d[c * 128:(c + 1) * 128, qs]
                        )
                    if s == 0:
                        nc.sync.dma_start(
                            wq_t[:], wq_d.rearrange("(c p) n -> p c n", p=128)
                        )
                        nc.sync.dma_start(mk_t[:], mk_d[:])
                        nc.sync.dma_start(bqm_t[:], bqm_d[:])
                        nc.sync.dma_start(
                            wv_t[:], wv_d.rearrange("(c p) n -> p c n", p=128)
                        )
                    elif s == 2:
                        nc.sync.dma_start(
                            wo_t[:], wo_d.rearrange("(c p) n -> p c n", p=128)
                        )

                # ---- per-rep tiles
                v1_t = vp.tile([128, ST, HPC * 65], BF16, tag="v1")
                with nc.allow_low_precision(reason="bf16 operands"):
                    nc.vector.tensor_copy(
                        v1_t[:].rearrange("p s (h c) -> p s h c", c=65)[:, :, :, 64],
                        ones_b[:, 0:64].rearrange("p (s h) -> p s h", s=ST),
                    )
                kt_t = [qk.tile([128, S], BF16, tag=f"kt{p}", name=f"kt{p}") for p in range(2)]
                qt_t = [qk.tile([128, QC, 2, QCW], BF16, tag=f"qt{p}", name=f"qt{p}") for p in range(2)]
                ctxt_t = [cp.tile([128, S], BF16, tag=f"ct{p}", name=f"ct{p}") for p in range(2)]

                # ---- PE warmup: ramp the p-state before real work arrives.
                for w in range(warmup):
                    wps = pp.tile([64, QCW], F32, tag="proj", bufs=2, name="warm")
                    nc.tensor.matmul(
                        wps[:], ones_b[:, 0:64],
                        warm_in[:], start=True, stop=True,
                    )

                # ---- projection units
                def kt_proj(pair, s):
                    qs = slice(s * QCW, (s + 1) * QCW)
                    kps = pp.tile([128, QCW], F32, tag="proj", bufs=2, name="kps")
                    for c in range(DC):
                        nc.tensor.matmul(
                            kps[:],
                            wk_t[:, c, pair * 128:(pair + 1) * 128],
                            xt_t[:, c, qs],
                            start=(c == 0),
                            stop=(c == DC - 1),
                        )
                    with nc.allow_low_precision(reason="bf16 operands"):
                        nc.vector.tensor_copy(kt_t[pair][:, qs], kps[:])

                def qt_proj(pair, s):
                    qs = slice(s * QCW, (s + 1) * QCW)
                    qps = pp.tile([128, QCW], F32, tag="proj", bufs=2, name="qps")
                    for c in range(DC):
                        nc.tensor.matmul(
                            qps[:],
                            wq_t[:, c, pair * 128:(pair + 1) * 128],
                            xt_t[:, c, qs],
                            start=(c == 0),
                            stop=(c == DC - 1),
                        )
                    for h in range(2):
                        hh = 2 * pair + h
                        with nc.allow_low_precision(reason="bf16 operands"):
                            nc.vector.tensor_scalar(
                                qt_t[pair][:, s, h, :], qps[:],
                                mk_t[:, h:h + 1], bqm_t[:, hh:hh + 1],
                                AluOpType.mult, AluOpType.add,
                            )

                def v_proj(st):
                    vps = pp.tile([128, HPC * HD], F32, tag="proj", bufs=2, name="vps")
                    for c in range(DC):
                        nc.tensor.matmul(
                            vps[:],
                            xt_t[:, c, st * 128:(st + 1) * 128],
                            wv_t[:, c, :],
                            start=(c == 0),
                            stop=(c == DC - 1),
                        )
                    with nc.allow_low_precision(reason="bf16 operands"):
                        nc.vector.tensor_copy(
                            v1_t[:, st, :].rearrange("p (h c) -> p h c", c=65)[:, :, 0:64],
                            vps[:].rearrange("p (h c) -> p h c", c=64),
                        )

                # ---- attention pieces
                def attn_rs(pair, qc, rs, ctx_ps, fillers=None, fe=None, pre=None):
                    n = 0
                    for r in rs:
                        sreg = pp.tile([128, 2 * QCW], F32, tag="sreg", bufs=2)
                        expt = ep.tile([128, 2 * QCW], BF16, tag="exp")
                        # one matmul per head: a matmul output must fit one
                        # PSUM bank (512 f32), so the two heads write the two
                        # banks of sreg separately.
                        for h in range(2):
                            nc.tensor.matmul(
                                sreg[:, h * QCW:(h + 1) * QCW],
                                kt_t[pair][:, r * 128:(r + 1) * 128],
                                qt_t[pair][:, qc, h, :],
                                start=True,
                                stop=True,
                            )
                        with nc.allow_low_precision(reason="bf16 exp output"):
                            nc.scalar.activation(expt[:], sreg[:], AF.Exp, scale=0.125)
                        for h in range(2):
                            hh = 2 * pair + h
                            nc.tensor.matmul(
                                ctx_ps[h][:],
                                v1_t[:, r, 65 * hh:65 * hh + 65],
                                expt[:, h * QCW:(h + 1) * QCW],
                                start=(r == 0),
                                stop=(r == KT - 1),
                            )
                        n += 1
                        if n == 1 and pre is not None:
                            # previous call's deferred normalization muls:
                            # their broadcast inputs are ready by now, so
                            # they retire without blocking the DVE queue.
                            norm_b(*pre)
                        if fillers and fe and n % fe == 0:
                            fillers.pop(0)()

                def norm_a(ctx_ps):
                    # drain PSUM into SBUF + compute broadcast reciprocals;
                    # the dependent muls are emitted later (norm_b) so the
                    # DVE queue never blocks on the Pool broadcast.
                    handles = []
                    for h in range(2):
                        c2 = c2p.tile([65, QCW], F32, tag="c2s", name=f"c2s{h}")
                        nc.vector.tensor_copy(c2[:], ctx_ps[h][:])
                        rsum = mp.tile([1, QCW], F32, tag="rsum")
                        nc.vector.reciprocal(rsum[:], c2[64:65, :])
                        handles.append((c2, rsum))
                    out = []
                    for h in range(2):
                        c2, rsum = handles[h]
                        bct = mp.tile([64, QCW], F32, tag="bc")
                        nc.gpsimd.partition_broadcast(bct[:], rsum[:])
                        out.append((c2, bct))
                    return out

                def norm_b(pair, qc, handles):
                    qs = slice(qc * QCW, (qc + 1) * QCW)
                    for h in range(2):
                        c2, bct = handles[h]
                        with nc.allow_low_precision(reason="bf16 ctx"):
                            nc.vector.tensor_mul(
                                ctxt_t[pair][64 * h:64 * (h + 1), qs],
                                c2[0:64, :],
                                bct[:],
                            )

                def new_ctx(pair):
                    return [
                        pp.tile([65, QCW], F32, tag="ctx", bufs=2, name=f"ctx{h}")
                        for h in range(2)
                    ]

                def outproj_unit(qc, sub, d2):
                    q0 = qc * QCW + sub * 128
                    ops = pp.tile([128, QCW], F32, tag="proj", bufs=2, name="ops")
                    for pair in range(2):
                        nc.tensor.matmul(
                            ops[:],
                            ctxt_t[pair][:, q0:q0 + 128],
                            wo_t[:, pair, d2 * 512:(d2 + 1) * 512],
                            start=(pair == 0),
                            stop=(pair == 1),
                        )
                    osb = op.tile([128, QCW], BF16, tag="osb")
                    with nc.allow_low_precision(reason="bf16 out"):
                        nc.vector.tensor_copy(osb[:], ops[:])
                    nc.sync.dma_start(
                        out_d[q0:q0 + 128, d2 * 512:(d2 + 1) * 512], osb[:]
                    )

                # ---- schedule ----
                # slab window: per-slab projections + attn(0,0) woven in.
                # kt1/qt1 of slabs 2,3 deferred as fillers for later calls.
                ctx00 = new_ctx(0)
                for s in range(QC):
                    kt_proj(0, s)
                    qt_proj(0, s)
                    for j in range(4):
                        v_proj(4 * s + j)
                    attn_rs(0, 0, range(4 * s, 4 * s + 4), ctx00)
                prev = (0, 0, norm_a(ctx00))

                # all pair-1 projections are deferred: they become PE filler
                # for the ACT(exp)-paced pair-0 attention stretch.
                deferred = [
                    (lambda f, p, s: lambda: f(p, s))(f, 1, s)
                    for s in range(QC)
                    for f in (kt_proj, qt_proj)
                ]
                for qc in range(1, QC):
                    ctx_ps = new_ctx(0)
                    # fe=5 -> 3 deferred units per call, spread evenly
                    attn_rs(0, qc, range(KT), ctx_ps, fillers=deferred, fe=5,
                            pre=prev)
                    prev = (0, qc, norm_a(ctx_ps))
                while deferred:  # safety: must be done before pair-1 scores
                    deferred.pop(0)()
                # pair 1 + output projection interleave: outproj(qc) units are
                # the fillers for attn(1, qc+1)'s ACT-paced stretch.
                pending = []
                for qc in range(QC):
                    ctx_ps = new_ctx(1)
                    attn_rs(1, qc, range(KT), ctx_ps, fillers=pending, fe=2,
                            pre=prev)
                    prev = (1, qc, norm_a(ctx_ps))
                    while pending:  # drain leftovers before reassigning
                        pending.pop(0)()
                    pending = [
                        (lambda q, su, d: lambda: outproj_unit(q, su, d))(qc, sub, d2)
                        for sub in range(4)
                        for d2 in range(2)
                    ]
                norm_b(*prev)
                for f in pending:
                    f()

    nc.compile()
    return nc


def _get_nc(repeat=1):
    key = repeat
    if key not in _CACHE:
        _CACHE[key] = _build(repeat)
    return _CACHE[key]


_MK2 = np.zeros((128, 2), np.float32)
_MK2[0:64, 0] = 1.0
_MK2[64:128, 1] = 1.0


def _bqm4(bqg):
    out = np.zeros((128, 4), np.float32)
    for h in range(4):
        hp = h % 2
        out[64 * hp:64 * hp + 64, h] = bqg[64 * h:64 * h + 64]
    return out


def _make_in_maps(query_input, Wq, bq, Wk, Wv, Wo):
    bf = ml_dtypes.bfloat16
    x = np.asarray(query_input, dtype=np.float32)
    in_maps = []
    for core in range(NCORES):
        b, g = divmod(core, NCORES // B)
        cs = slice(g * HPC * HD, (g + 1) * HPC * HD)
        in_maps.append({
            "xt": np.ascontiguousarray(x[b].T).astype(bf),
            "wq": np.ascontiguousarray(Wq[:, cs]).astype(bf),
            "wk": np.ascontiguousarray(Wk[:, cs]).astype(bf),
            "wv": np.ascontiguousarray(Wv[:, cs]).astype(bf),
            "wo": np.ascontiguousarray(Wo[cs, :]).astype(bf),
            "mk2": _MK2,
            "bqm4": np.ascontiguousarray(_bqm4(bq[cs])),
        })
    return in_maps


def kernel(query_input, Wq, bq, Wk, bk, Wv, bv, Wo, bo):
    from concourse.bass_utils import run_bass_kernel_spmd

    Wq = np.asarray(Wq, np.float32)
    Wk = np.asarray(Wk, np.float32)
    Wv = np.asarray(Wv, np.float32)
    Wo = np.asarray(Wo, np.float32)
    bq = np.asarray(bq, np.float32)
    bv = np.asarray(bv, np.float32)
    bo = np.asarray(bo, np.float32)

    nc = _get_nc()
    in_maps = _make_in_maps(query_input, Wq, bq, Wk, Wv, Wo)
    res = run_bass_kernel_spmd(nc, in_maps, core_ids=list(range(NCORES)))

    gpc = NCORES // B  # groups per batch
    out = np.zeros((B, S, D), np.float32)
    for core in range(NCORES):
        b = core // gpc
        out[b] += np.asarray(res.results[core]["out_p"], dtype=np.float32)
    # bv correction (exact) + bo, applied once on the full output
    out += (bv @ Wo + bo)[None, None, :]
    return out
